# revision 13
# baseline (speedup 1.0000x reference)
"""DIFF-Transformer block kernel for 8 Trainium2 NeuronCores.

Sharding: core c handles batch b=c//2 and query-token-half t=c%2.
Each core receives x for its batch TRANSPOSED ([768, 1024] feature-major,
own token half first), computes LN1 + dual QKV + differential attention +
RMSNorm + proj + residual + LN2 + MLP for its 512 query tokens, and writes
the [768, 512] transposed output slice.  K/V are computed over the full
1024 tokens of the batch on both cores of a pair (duplicated work instead
of a collective).

All on-chip compute is in a transposed ([feature, token]) layout so no
transposes are ever needed:
  - qT/kT come out of the QKV matmul directly as [head_dim, token],
  - scores are built as sT[m, n] (keys on partitions), exp'd in place,
  - o^T accumulates via lhsT = [v | ones] so softmax denominators fall out
    of the same matmul (row 64),
  - a1 - lam*a2 normalization uses RMSNorm scale-invariance so only one
    per-token scale (s = lam*sum1/sum2) is ever applied.

Affine folds done on the host: ln1_w/b into qkv weights/biases, the
1/sqrt(hd) scale into the q weights, rms_w into proj, ln2_w/b into fc1.
Matmul operands are bf16 (fp32 accumulation in PSUM); the residual trunk
stays fp32 end-to-end.
"""

import os
import sys

import numpy as np

for _p in ("/opt/trn_rl_repo",):
    if os.path.isdir(_p) and _p not in sys.path:
        sys.path.insert(0, _p)

import ml_dtypes  # noqa: E402

import concourse.bass as bass  # noqa: E402
import concourse.mybir as mybir  # noqa: E402
from concourse.bass_utils import run_bass_kernel_spmd  # noqa: E402
from concourse.tile import TileContext  # noqa: E402
from concourse.vector_clock import ScopedClock  # noqa: E402


class _SplitDrainTC(TileContext):
    """TileContext whose kernel-tail drain spreads its semaphore waits over
    single-wait nops: the walrus build in this container rejects
    instructions carrying more than a couple of sync waits
    ("Too many sync wait commands" in CoreV3 codegen)."""

    def _drain_and_barrier(self, tick_clock, wait_clock):
        nc = self.nc
        probe = nc.sync.nop()
        wait_clock.add_sem_waits(
            probe.ins, ScopedClock({None: tick_clock.global_clock})
        )
        si = probe.ins.sync_info
        waits = list(si.on_wait) if si is not None else []
        if len(waits) > 1:
            si.on_wait = waits[:1]
            probe.ins.sync_info = si
            for i in range(1, len(waits)):
                nop = nc.sync.nop()
                nop.ins.sync_info = mybir.SyncInfo(on_wait=[waits[i]],
                                                   on_update=[])
        nc.sync.drain()
        nc.all_engine_barrier()
        popped = nc._tile_sem_poison_stack.pop()
        assert popped is self._sem_poison
        nc.clear_and_free_semaphores(list(self.sems.allocated().values()))
        nc.all_engine_barrier()

BF = ml_dtypes.bfloat16

B, N, D, H, HD = 4, 1024, 768, 12, 64
MLP = 4 * D
P = 128
DT = D // P            # 6 d-tiles
MT = MLP // P          # 24 mlp tiles
NQ = 512               # query tokens per core
NK = 1024              # key tokens per core
LAMBDA_INIT = 0.1

F32 = mybir.dt.float32
BF16 = mybir.dt.bfloat16
AF = mybir.ActivationFunctionType

LAST_EXEC_NS = None
_CACHE = {}


def _split_sync_waits(nc, max_waits=1):
    """Walrus in this container caps sync waits per instruction; hoist extra
    waits onto same-engine nops inserted right before the instruction."""
    for f in nc.m.functions:
        for b in f.blocks:
            out = []
            changed = False
            for inst in b.instructions:
                si = inst.sync_info
                waits = list(si.on_wait) if si is not None else []
                if len(waits) > max_waits:
                    changed = True
                    for j, w in enumerate(waits[max_waits:]):
                        nop = mybir.InstNoOp(name=f"{inst.name}-wsplit{j}",
                                             ins=[], outs=[],
                                             engine=inst.engine)
                        nop.sync_info = mybir.SyncInfo(on_wait=[w],
                                                       on_update=[])
                        out.append(nop)
                    si.on_wait = waits[:max_waits]
                    inst.sync_info = si
                out.append(inst)
            if changed:
                b.instructions = out


def _layernorm_T(nc, tc, pools, x_bf, out_bf, n_tok, ones_bf, ones1_bf, eps):
    """LayerNorm over the partition (feature) axis of x_bf [128, DT, n_tok].

    Writes the normalized (zero-mean, unit-var; no affine) result to out_bf.
    Stats are computed with ones-matmuls; per-token stats are broadcast
    across partitions with K=1 matmuls.
    """
    ps_stat, ps_bc, sm = pools
    for j in range(n_tok // 512):
        sl = slice(512 * j, 512 * j + 512)
        mean_ps = ps_stat.tile([1, 512], F32, tag="stat", name="mean_ps")
        for d in range(DT):
            nc.tensor.matmul(mean_ps, ones_bf, x_bf[:, d, sl],
                             start=(d == 0), stop=(d == DT - 1))
        ssq_ps = ps_stat.tile([1, 512], F32, tag="stat", name="ssq_ps")
        for d in range(DT):
            sq = sm.tile([128, 512], BF16, tag="sq", name="sq")
            nc.scalar.square(sq, x_bf[:, d, sl])
            nc.tensor.matmul(ssq_ps, ones_bf, sq,
                             start=(d == 0), stop=(d == DT - 1))
        mean_sb = sm.tile([1, 512], BF16, tag="mrow", name="mean_sb")
        nc.vector.tensor_scalar_mul(mean_sb, mean_ps, 1.0 / D)
        musq = sm.tile([1, 512], F32, tag="musq", name="musq")
        nc.vector.tensor_mul(musq, mean_sb, mean_sb)
        var = sm.tile([1, 512], F32, tag="var", name="var")
        nc.vector.tensor_scalar_mul(var, ssq_ps, 1.0 / D)
        nc.vector.tensor_sub(var, var, musq)
        std = sm.tile([1, 512], F32, tag="std", name="std")
        nc.scalar.activation(std, var, AF.Sqrt, bias=eps[0:1], scale=1.0)
        rstd = sm.tile([1, 512], BF16, tag="rrow", name="rstd")
        with nc.allow_low_precision(reason="rstd row feeds bf16 broadcast"):
            nc.vector.reciprocal(rstd, std)

        mb_ps = ps_bc.tile([128, 512], F32, tag="bc", name="mb_ps")
        nc.tensor.matmul(mb_ps, ones1_bf, mean_sb, start=True, stop=True)
        rb_ps = ps_bc.tile([128, 512], F32, tag="bc", name="rb_ps")
        nc.tensor.matmul(rb_ps, ones1_bf, rstd, start=True, stop=True)
        mb = sm.tile([128, 512], BF16, tag="mb", name="mb")
        nc.scalar.copy(mb, mb_ps)
        rb = sm.tile([128, 512], BF16, tag="rb", name="rb")
        nc.scalar.copy(rb, rb_ps)
        for d in range(DT):
            xc = sm.tile([128, 512], BF16, tag="xc", name="xc")
            nc.vector.tensor_sub(xc, x_bf[:, d, sl], mb)
            nc.vector.tensor_mul(out_bf[:, d, sl], xc, rb)


def _build(lam):
    """Build the SPMD Bass program. lam: tuple of 12 per-head floats."""
    nc = bass.Bass()
    dp = nc.declare_dram_parameter
    xT_d = dp("xT", [D, NK], F32, False)
    w1_d = dp("w1T", [D, 3 * D], BF16, False)     # [d, q1|k1|v1] (ln1_w, scale folded)
    w2_d = dp("w2T", [D, 2 * D], BF16, False)     # [d, q2|k2]
    pj_d = dp("pjT", [D, D], BF16, False)         # (proj_w * rms_w).T
    f1_d = dp("f1T", [D, MLP], BF16, False)       # (fc1_w * ln2_w).T
    f2_d = dp("f2T", [MLP, D], BF16, False)
    qb1_d = dp("qb1", [12, 128], F32, False)      # q1|k1 bias per c-tile (from ln1_b)
    qb2_d = dp("qb2", [12, 128], F32, False)      # q2|k2 bias
    vb_d = dp("vb", [1, D], BF16, False)          # v1 bias row
    pb_d = dp("pb", [DT, 128], F32, False)        # proj_b
    b1_d = dp("b1", [MT, 128], F32, False)        # fc1 bias (ln2_b folded)
    b2_d = dp("b2", [DT, 128], F32, False)        # fc2 bias
    out_d = dp("out", [D, NQ], F32, True)

    with _SplitDrainTC(nc) as tc:
        with tc.tile_pool(name="big", bufs=1) as big, \
             tc.tile_pool(name="const", bufs=1) as const:
            # ---- constants ----
            ones_bf = const.tile([128, 1], BF16, name="ones_bf")
            nc.vector.memset(ones_bf, 1.0)
            ones1_bf = const.tile([1, 128], BF16, name="ones1_bf")
            nc.vector.memset(ones1_bf, 1.0)
            zero_f = const.tile([128, 1], F32, name="zero_f")
            nc.vector.memset(zero_f, 0.0)
            nc.const_aps.aps[(F32, 0.0)] = zero_f
            eps5 = const.tile([128, 1], F32, name="eps5")
            nc.vector.memset(eps5, 1e-5)
            eps6 = const.tile([128, 1], F32, name="eps6")
            nc.vector.memset(eps6, 1e-6)
            qb1_sb = const.tile([128, 12], F32, name="qb1_sb")
            nc.sync.dma_start(qb1_sb, qb1_d.rearrange("t p -> p t"))
            qb2_sb = const.tile([128, 12], F32, name="qb2_sb")
            nc.sync.dma_start(qb2_sb, qb2_d.rearrange("t p -> p t"))
            pb_sb = const.tile([128, DT], F32, name="pb_sb")
            nc.sync.dma_start(pb_sb, pb_d.rearrange("t p -> p t"))
            b1_sb = const.tile([128, MT], F32, name="b1_sb")
            nc.sync.dma_start(b1_sb, b1_d.rearrange("t p -> p t"))
            b2_sb = const.tile([128, DT], F32, name="b2_sb")
            nc.sync.dma_start(b2_sb, b2_d.rearrange("t p -> p t"))
            vbrow_sb = const.tile([1, D], BF16, name="vbrow_sb")
            nc.sync.dma_start(vbrow_sb, vb_d[:, :])

            # v bias broadcast to all 128 token-partitions (once)
            vb_sb = const.tile([128, D], BF16, name="vb_sb")

            # ---- persistent activations ----
            xT_sb = big.tile([128, DT, NK], F32, name="xT_sb")
            nc.sync.dma_start(xT_sb, xT_d.rearrange("(t p) n -> p t n", p=P))
            x_bf = big.tile([128, DT, NK], BF16, name="x_bf")
            for d in range(DT):
                nc.vector.tensor_copy(x_bf[:, d], xT_sb[:, d])
            hT = big.tile([128, DT, NK], BF16, name="hT")
            q1T = big.tile([128, DT, NQ], BF16, name="q1T")
            q2T = big.tile([128, DT, NQ], BF16, name="q2T")
            k1T = big.tile([128, DT, NK], BF16, name="k1T")
            k2T = big.tile([128, DT, NK], BF16, name="k2T")
            vaug = big.tile([128, 8, H, HD + 1], BF16, name="vaug")
            nc.gpsimd.memset(vaug, 1.0)
            oT = big.tile([128, DT, NQ], BF16, name="oT")
            x2T = big.tile([128, DT, NQ], F32, name="x2T")
            x2_bf = big.tile([128, DT, NQ], BF16, name="x2_bf")
            h2T = big.tile([128, DT, NQ], BF16, name="h2T")

            # ================= Phase A: LN1 =================
            with tc.tile_pool(name="psA", bufs=4, space="PSUM") as ps_stat, \
                 tc.tile_pool(name="psAb", bufs=2, space="PSUM") as ps_bc, \
                 tc.tile_pool(name="smA", bufs=2) as smA:
                # broadcast v bias while PE is otherwise idle
                vbb_ps = ps_bc.tile([128, D], F32, tag="vbb", bufs=1,
                                    name="vbb_ps")
                nc.tensor.matmul(vbb_ps[:, 0:512], ones1_bf, vbrow_sb[:, 0:512],
                                 start=True, stop=True)
                nc.tensor.matmul(vbb_ps[:, 512:768], ones1_bf,
                                 vbrow_sb[:, 512:768], start=True, stop=True)
                nc.scalar.copy(vb_sb, vbb_ps)
                _layernorm_T(nc, tc, (ps_stat, ps_bc, smA), x_bf, hT, NK,
                             ones_bf, ones1_bf, eps5)

            # ================= Phase B: QKV =================
            with tc.tile_pool(name="wq", bufs=1) as wq, \
                 tc.tile_pool(name="psB", bufs=6, space="PSUM") as psB:
                w1_sb = wq.tile([128, DT, 3 * D], BF16, name="w1_sb")
                nc.sync.dma_start(w1_sb, w1_d.rearrange("(t p) c -> p t c", p=P))
                w2_sb = wq.tile([128, DT, 2 * D], BF16, name="w2_sb")
                nc.sync.dma_start(w2_sb, w2_d.rearrange("(t p) c -> p t c", p=P))

                def qkv_ct(dst, w_sb, ct, bias_sb, bidx, tok_sl):
                    ps = psB.tile([128, 512], F32, tag="ps", name="qkv_ps")
                    ntok = tok_sl.stop - tok_sl.start
                    for d in range(DT):
                        nc.tensor.matmul(ps[:, :ntok],
                                         w_sb[:, d, ct * P:(ct + 1) * P],
                                         hT[:, d, tok_sl],
                                         start=(d == 0), stop=(d == DT - 1))
                    nc.scalar.activation(dst, ps[:, :ntok], AF.Identity,
                                         bias=bias_sb[:, bidx:bidx + 1],
                                         scale=1.0)

                for ct in range(DT):  # q1, q2 (own tokens only)
                    qkv_ct(q1T[:, ct], w1_sb, ct, qb1_sb, ct, slice(0, NQ))
                    qkv_ct(q2T[:, ct], w2_sb, ct, qb2_sb, ct, slice(0, NQ))
                for ct in range(DT):  # k1, k2 (all tokens)
                    for j in range(2):
                        sl = slice(512 * j, 512 * j + 512)
                        qkv_ct(k1T[:, ct, sl], w1_sb, DT + ct, qb1_sb, DT + ct, sl)
                        qkv_ct(k2T[:, ct, sl], w2_sb, DT + ct, qb2_sb, DT + ct, sl)
                # v1 in token-major layout, into the augmented [v|1] tile
                for m in range(8):
                    for cc in range(2):
                        psv = psB.tile([128, 384], F32, tag="ps", name="v_ps")
                        for d in range(DT):
                            nc.tensor.matmul(
                                psv, hT[:, d, m * P:(m + 1) * P],
                                w1_sb[:, d, 2 * D + cc * 384: 2 * D + cc * 384 + 384],
                                start=(d == 0), stop=(d == DT - 1))
                        nc.vector.tensor_add(
                            vaug[:, m, 6 * cc:6 * cc + 6, 0:HD],
                            psv.rearrange("p (h e) -> p h e", e=HD),
                            vb_sb[:, cc * 384:cc * 384 + 384].rearrange(
                                "p (h e) -> p h e", e=HD))

            # ================= Phase C: differential attention =============
            with tc.tile_pool(name="psC1", bufs=1, space="PSUM") as psS1, \
                 tc.tile_pool(name="psC2", bufs=1, space="PSUM") as psS2, \
                 tc.tile_pool(name="psCo", bufs=3, space="PSUM") as psO, \
                 tc.tile_pool(name="psCb", bufs=1, space="PSUM") as psCb, \
                 tc.tile_pool(name="esb", bufs=12) as esb, \
                 tc.tile_pool(name="smC", bufs=2) as smC:
                for h in range(H):
                    t, r0 = h // 2, HD * (h % 2)
                    elists = ([], [])
                    for si, (kT, qT, spool) in enumerate(
                            ((k1T, q1T, psS1), (k2T, q2T, psS2))):
                        for mp in range(4):
                            ps = spool.tile([128, 2, 512], F32, tag="s",
                                            name="score_ps")
                            for j in range(2):
                                m0 = (mp * 2 + j) * P
                                nc.tensor.matmul(
                                    ps[:, j],
                                    kT[r0:r0 + HD, t, m0:m0 + P],
                                    qT[r0:r0 + HD, t, :],
                                    start=True, stop=True)
                            e = esb.tile([128, 2, 512], BF16, tag="e", name="e")
                            nc.scalar.activation(e, ps, AF.Exp)
                            elists[si].append(e)
                    o1p = psO.tile([HD + 1, 512], F32, tag="o", name="o1p")
                    o2p = psO.tile([HD + 1, 512], F32, tag="o", name="o2p")
                    for m in range(8):
                        va = vaug[:, m, h, :]
                        nc.tensor.matmul(o1p, va, elists[0][m // 2][:, m % 2],
                                         start=(m == 0), stop=(m == 7))
                        nc.tensor.matmul(o2p, va, elists[1][m // 2][:, m % 2],
                                         start=(m == 0), stop=(m == 7))
                    # w = o1 - (lam*sum1/sum2) * o2 ; global 1/sum1 cancels in RMSNorm
                    r2 = smC.tile([1, 512], F32, tag="r2", name="r2")
                    nc.vector.reciprocal(r2, o2p[HD:HD + 1, :])
                    srow = smC.tile([1, 512], BF16, tag="srow", name="srow")
                    nc.vector.tensor_mul(srow, o1p[HD:HD + 1, :], r2)
                    nc.vector.tensor_scalar_mul(srow, srow, float(lam[h]))
                    sb_ps = psCb.tile([HD, 512], F32, tag="bc2", name="sb_ps")
                    nc.tensor.matmul(sb_ps, ones1_bf[:, 0:HD], srow,
                                     start=True, stop=True)
                    sbb = smC.tile([HD, 512], F32, tag="sbb", name="sbb")
                    nc.scalar.copy(sbb, sb_ps)
                    tmpc = smC.tile([HD, 512], F32, tag="tmpc", name="tmpc")
                    nc.vector.tensor_mul(tmpc, o2p[0:HD, :], sbb)
                    nc.vector.tensor_sub(oT[r0:r0 + HD, t, :], o1p[0:HD, :], tmpc)

            # ================= Phase D: RMSNorm + proj + residual ==========
            with tc.tile_pool(name="psD", bufs=1, space="PSUM") as psDs, \
                 tc.tile_pool(name="psDb", bufs=1, space="PSUM") as psDb, \
                 tc.tile_pool(name="psDa", bufs=2, space="PSUM") as psDa, \
                 tc.tile_pool(name="wpj", bufs=1) as wpj, \
                 tc.tile_pool(name="smD", bufs=2) as smD:
                pj_sb = wpj.tile([128, DT, D], BF16, name="pj_sb")
                nc.sync.dma_start(pj_sb, pj_d.rearrange("(t p) c -> p t c", p=P))
                ssq = psDs.tile([1, 512], F32, tag="ssq", name="ssq")
                for d in range(DT):
                    sq2 = smD.tile([128, 512], BF16, tag="sq2", name="sq2")
                    nc.scalar.square(sq2, oT[:, d])
                    nc.tensor.matmul(ssq, ones_bf, sq2,
                                     start=(d == 0), stop=(d == DT - 1))
                std2 = smD.tile([1, 512], F32, tag="std2", name="std2")
                nc.scalar.activation(std2, ssq, AF.Sqrt, bias=eps6[0:1],
                                     scale=1.0 / D)
                rstd2 = smD.tile([1, 512], BF16, tag="rstd2", name="rstd2")
                with nc.allow_low_precision(reason="rstd row feeds bf16 broadcast"):
                    nc.vector.reciprocal(rstd2, std2)
                rb2_ps = psDb.tile([128, 512], F32, tag="bcD", name="rb2_ps")
                nc.tensor.matmul(rb2_ps, ones1_bf, rstd2, start=True, stop=True)
                rb2 = smD.tile([128, 512], BF16, tag="rb2", name="rb2")
                nc.scalar.copy(rb2, rb2_ps)
                orm = smD.tile([128, DT, 512], BF16, tag="orm", bufs=1, name="orm")
                for d in range(DT):
                    nc.vector.tensor_mul(orm[:, d], oT[:, d], rb2)
                for ct in range(DT):
                    ps = psDa.tile([128, 512], F32, tag="at", name="at_ps")
                    for d in range(DT):
                        nc.tensor.matmul(ps, pj_sb[:, d, ct * P:(ct + 1) * P],
                                         orm[:, d],
                                         start=(d == 0), stop=(d == DT - 1))
                    tmp2 = smD.tile([128, 512], F32, tag="tmp2", name="tmp2")
                    nc.scalar.activation(tmp2, ps, AF.Identity,
                                         bias=pb_sb[:, ct:ct + 1], scale=1.0)
                    nc.vector.tensor_add(x2T[:, ct], tmp2, xT_sb[:, ct, 0:NQ])
                    nc.vector.tensor_copy(x2_bf[:, ct], x2T[:, ct])

            # ================= Phase E: LN2 =================
            with tc.tile_pool(name="psE", bufs=2, space="PSUM") as ps_stat2, \
                 tc.tile_pool(name="psEb", bufs=2, space="PSUM") as ps_bc2, \
                 tc.tile_pool(name="smE", bufs=2) as smE:
                _layernorm_T(nc, tc, (ps_stat2, ps_bc2, smE), x2_bf, h2T, NQ,
                             ones_bf, ones1_bf, eps5)

            # ================= Phase F: MLP + residual =================
            with tc.tile_pool(name="wf1", bufs=1) as wf1, \
                 tc.tile_pool(name="wf2", bufs=3) as wf2, \
                 tc.tile_pool(name="psFg", bufs=2, space="PSUM") as psFg, \
                 tc.tile_pool(name="psFa", bufs=1, space="PSUM") as psFa, \
                 tc.tile_pool(name="smF", bufs=3) as smF:
                f1_sb = wf1.tile([128, DT, MLP], BF16, name="f1_sb")
                nc.sync.dma_start(f1_sb, f1_d.rearrange("(t p) c -> p t c", p=P))
                accs = [psFa.tile([128, 512], F32, tag=f"acc{i}", name=f"acc{i}")
                        for i in range(DT)]
                for mt in range(MT):
                    gp = psFg.tile([128, 512], F32, tag="g", name="g_ps")
                    for d in range(DT):
                        nc.tensor.matmul(gp, f1_sb[:, d, mt * P:(mt + 1) * P],
                                         h2T[:, d],
                                         start=(d == 0), stop=(d == DT - 1))
                    gsb = smF.tile([128, 512], BF16, tag="gsb", name="gsb")
                    nc.scalar.activation(gsb, gp, AF.Gelu,
                                         bias=b1_sb[:, mt:mt + 1], scale=1.0)
                    f2t = wf2.tile([128, D], BF16, tag="f2", name="f2t")
                    nc.sync.dma_start(f2t, f2_d[mt * P:(mt + 1) * P, :])
                    for ct in range(DT):
                        nc.tensor.matmul(accs[ct], f2t[:, ct * P:(ct + 1) * P],
                                         gsb, start=(mt == 0), stop=(mt == MT - 1))
                for ct in range(DT):
                    tmp3 = smF.tile([128, 512], F32, tag="tmp3", name="tmp3")
                    nc.scalar.activation(tmp3, accs[ct], AF.Identity,
                                         bias=b2_sb[:, ct:ct + 1], scale=1.0)
                    osb = smF.tile([128, 512], F32, tag="osb", name="osb")
                    nc.vector.tensor_add(osb, tmp3, x2T[:, ct])
                    nc.sync.dma_start(out_d[ct * P:(ct + 1) * P, :], osb)
    _split_sync_waits(nc)
    return nc


def _prep(inputs):
    f = lambda k: np.asarray(inputs[k], np.float32)
    x = f("x")
    ln1_w, ln1_b = f("ln1_w"), f("ln1_b")
    qkv1_w, qkv2_w = f("qkv1_w"), f("qkv2_w")
    proj_w, proj_b = f("proj_w"), f("proj_b")
    rms_w = f("rms_w")
    lam1, lam2 = f("lam1").reshape(H), f("lam2").reshape(H)
    ln2_w, ln2_b = f("ln2_w"), f("ln2_b")
    fc1_w, fc1_b = f("fc1_w"), f("fc1_b")
    fc2_w, fc2_b = f("fc2_w"), f("fc2_b")

    lam = tuple(float(v) for v in (lam1 - lam2 + LAMBDA_INIT))
    scale = HD ** -0.5

    w1f = qkv1_w * ln1_w[None, :]
    w2f = qkv2_w[:2 * D] * ln1_w[None, :]
    qb1 = qkv1_w @ ln1_b
    qb2 = (qkv2_w @ ln1_b)[:2 * D]
    w1f[0:D] *= scale
    qb1[0:D] *= scale
    w2f[0:D] *= scale
    qb2[0:D] *= scale

    shared = {
        "w1T": np.ascontiguousarray(w1f.T).astype(BF),
        "w2T": np.ascontiguousarray(w2f.T).astype(BF),
        "pjT": np.ascontiguousarray((proj_w * rms_w[None, :]).T).astype(BF),
        "f1T": np.ascontiguousarray((fc1_w * ln2_w[None, :]).T).astype(BF),
        "f2T": np.ascontiguousarray(fc2_w.T).astype(BF),
        "qb1": np.ascontiguousarray(qb1[:2 * D].reshape(12, 128), np.float32),
        "qb2": np.ascontiguousarray(qb2.reshape(12, 128), np.float32),
        "vb": np.ascontiguousarray(qb1[2 * D:].reshape(1, D)).astype(BF),
        "pb": np.ascontiguousarray(proj_b.reshape(DT, 128), np.float32),
        "b1": np.ascontiguousarray((fc1_b + fc1_w @ ln2_b).reshape(MT, 128),
                                   np.float32),
        "b2": np.ascontiguousarray(fc2_b.reshape(DT, 128), np.float32),
    }
    in_maps = []
    for c in range(8):
        b, t = c // 2, c % 2
        xb = x[b]
        xr = np.concatenate([xb[t * NQ:(t + 1) * NQ],
                             xb[(1 - t) * NQ:(2 - t) * NQ]], axis=0)
        m = dict(shared)
        m["xT"] = np.ascontiguousarray(xr.T, np.float32)
        in_maps.append(m)
    return lam, in_maps


def kernel(**inputs):
    global LAST_EXEC_NS
    lam, in_maps = _prep(inputs)
    if lam not in _CACHE:
        _CACHE[lam] = _build(lam)
    nc = _CACHE[lam]
    trace = bool(int(os.environ.get("BASS_KERNEL_TRACE", "0")))
    res = run_bass_kernel_spmd(nc, in_maps, list(range(8)), trace=trace)
    LAST_EXEC_NS = res.exec_time_ns
    y = np.empty((B, N, D), np.float32)
    for c in range(8):
        b, t = c // 2, c % 2
        y[b, t * NQ:(t + 1) * NQ] = np.asarray(res.results[c]["out"]).T
    return y
